# revision 8
# baseline (speedup 1.0000x reference)
"""Tensor-parallel DeepSpeed-style self-attention block on 8 TRN2 NeuronCores.

Strategy (head-sharded QKV/attention + all-to-all + token-sharded output GEMM):
  - LayerNorm gamma/beta are folded into the QKV weight/bias on host:
      qkv = z @ (norm_w[:,None]*W) + (norm_b @ W + qkvb),  z=(x-mu)*istd
  - Each core owns 2 of 16 heads. z (bf16) is computed replicated, bounced
    through DRAM, and re-loaded transposed via the DMA x-bar, giving z^T
    without TensorE transposes. From z^T the core computes Q^T, K^T
    (transposed layout) and V (natural layout) for its heads, all tokens.
  - Causal attention per (batch, head): scores = Q^T.T @ K^T in PSUM over
    key-blocks up to the diagonal; triangular -1e9 mask added to the
    diagonal 128x128 block; exp on ScalarE (no max subtraction - scores
    are bounded ~6 sigma for this distribution) with accum_out row-sums.
    p is zero-padded to the 512 boundary, bounced through DRAM, and
    re-loaded transposed (x-bar) as p^T [k,128 x q,512] chunks, so
    ctx^T = V.T @ p^T accumulates directly in [d, q] layout with N=512
    matmuls. Row-sum reciprocals ride a tiny DRAM bounce to become a
    [1,512] row, are partition-broadcast, and normalize ctx^T on DVE.
  - AllToAll (one per batch, bf16, 1MB) redistributes ctx^T from
    head-sharded to token-sharded: each core ends with all 16 heads for
    its 256-token slice of each batch.
  - Output GEMM: full attn_ow (replicated, cast bf16) x token shard.
    Each core writes a [512, 2048] f32 output shard; host concatenates.
"""

import sys

if "/opt/trn_rl_repo" not in sys.path:
    sys.path.insert(0, "/opt/trn_rl_repo")

# --- shim antenv.axon_hooks (missing in this image) so trace=True can NTFF-profile ---
import types, ctypes, contextlib


def _make_ntff_hook(so_path="/opt/axon/libaxon_pjrt.so"):
    try:
        lib = ctypes.CDLL(so_path)
    except OSError:
        return None
    if not hasattr(lib, "axon_start_nrt_profile"):
        return None
    lib.axon_start_nrt_profile.argtypes = [ctypes.POINTER(ctypes.c_int64), ctypes.c_size_t]
    lib.axon_start_nrt_profile.restype = ctypes.c_int64
    lib.axon_stop_nrt_profile.argtypes = [ctypes.c_char_p]
    lib.axon_stop_nrt_profile.restype = ctypes.c_int64

    @contextlib.contextmanager
    def _hook(output_dir, device_ids):
        import jax

        jax.devices()
        if device_ids:
            ids = (ctypes.c_int64 * len(device_ids))(*device_ids)
            rc = lib.axon_start_nrt_profile(ids, len(device_ids))
        else:
            rc = lib.axon_start_nrt_profile(None, 0)
        if rc != 0:
            raise RuntimeError(f"axon_start_nrt_profile rc={rc}")
        try:
            yield
        finally:
            n = lib.axon_stop_nrt_profile(str(output_dir).encode())
            if n < 0:
                raise RuntimeError(f"axon_stop_nrt_profile rc={n}")

    return _hook


if "antenv.axon_hooks" not in sys.modules:
    _m = types.ModuleType("antenv.axon_hooks")
    _m.get_axon_ntff_profile_hook = lambda: _make_ntff_hook()
    sys.modules["antenv.axon_hooks"] = _m
# --- end shim ---

import numpy as np
import ml_dtypes  # noqa: F401  (bf16 numpy dtype registration)

from concourse import bacc, tile, mybir
from concourse.masks import make_causal_mask

B, S, HID = 2, 2048, 2048
HEADS = 16
HD = 128  # head dim
T = B * S  # 4096 tokens
N_CORES = 8
HPC = HEADS // N_CORES  # 2 heads per core
EPS = 1e-6
SCALE = 1.0 / float(np.sqrt(HD))
NEG = -1e9

F32 = mybir.dt.float32
BF16 = mybir.dt.bfloat16

TOKB = 128  # token block (partition dim)
SBK = 512  # superblock of tokens
N_SB = T // SBK  # 8
N_TB = SBK // TOKB  # 4
N_CC = HID // 128  # 16 contraction chunks
N_QB = S // TOKB  # 16 q-blocks per batch
N_G = S // SBK  # 4 q-groups of 512 per batch
TOK_SHARD = S // N_CORES  # 256 tokens per (batch, core) after A2A


def _build(apply_mask: bool):
    nc = bacc.Bacc("TRN2", target_bir_lowering=False, debug=False, num_devices=N_CORES)

    inp = nc.dram_tensor("input", [T, HID], F32, kind="ExternalInput").ap()
    wqkv = nc.dram_tensor("qkvw", [HID, 3 * HPC * HD], F32, kind="ExternalInput").ap()
    qkb = nc.dram_tensor("qkb", [128, 2 * HPC], F32, kind="ExternalInput").ap()
    vb = nc.dram_tensor("vb", [1, HPC * HD], F32, kind="ExternalInput").ap()
    ow = nc.dram_tensor("ow", [HID, HID], F32, kind="ExternalInput").ap()
    out = nc.dram_tensor("out", [B * TOK_SHARD, HID], F32, kind="ExternalOutput").ap()
    if apply_mask:
        imask = nc.dram_tensor("imask", [1, B * S], F32, kind="ExternalInput").ap()

    # DRAM bounce buffers (separate tensors keep dep tracking fine-grained)
    z_dram = [nc.dram_tensor(f"zd{i}", [SBK, HID], BF16).ap() for i in range(N_SB)]
    p_dram = [[nc.dram_tensor(f"pd{par}_{g}", [SBK, SBK * (g + 1)], BF16).ap() for g in range(N_G)] for par in range(2)]
    rs_dram = [[nc.dram_tensor(f"rs{par}_{g}", [1, SBK], F32).ap() for g in range(N_G)] for par in range(2)]
    cc_in = [nc.dram_tensor(f"cc_in{b}", [N_CORES, HPC * HD, TOK_SHARD], BF16).ap() for b in range(B)]
    cc_out = [nc.dram_tensor(f"cc_out{b}", [N_CORES, HPC * HD, TOK_SHARD], BF16).ap() for b in range(B)]

    with tile.TileContext(nc) as tc:
        with tc.tile_pool(name="persist", bufs=1) as pers:
            causal = pers.tile([128, 128], F32)
            make_causal_mask(nc, causal[:], mask_val=NEG)
            eps_t = pers.tile([128, 1], F32)
            nc.gpsimd.memset(eps_t[:], EPS)
            qkb_sb = pers.tile([128, 2 * HPC], F32)
            nc.sync.dma_start(out=qkb_sb[:], in_=qkb[:])
            vbf = pers.tile([1, HPC * HD], F32)
            nc.sync.dma_start(out=vbf[:], in_=vb[:])
            vb_bc = pers.tile([128, HPC * HD], F32)
            nc.gpsimd.partition_broadcast(vb_bc[:], vbf[:])

            qT = pers.tile([128, HPC, T], BF16)  # [d, head, tok]
            kT = pers.tile([128, HPC, T], BF16)
            v_sb = pers.tile([128, T // 128, HPC * HD], BF16)  # [tok128, blk, hcol]

            if apply_mask:
                msk = pers.tile([128, B, S], F32)
                mrow = pers.tile([1, B * S], F32)
                nc.sync.dma_start(out=mrow[:], in_=imask[:])
                for b in range(B):
                    nc.gpsimd.partition_broadcast(msk[:, b, :], mrow[:, b * S : (b + 1) * S])

            # ---------------- Phase A: LN -> z -> (DRAM xbar) -> z^T -> QKV GEMM ----------------
            with (
                tc.tile_pool(name="pa_w", bufs=1) as paw,
                tc.tile_pool(name="pa_x", bufs=3) as px,
                tc.tile_pool(name="pa_st", bufs=6) as pst,
                tc.tile_pool(name="pa_z", bufs=3) as pz,
                tc.tile_pool(name="pa_zT", bufs=2) as pzT,
                tc.tile_pool(name="pa_cast", bufs=2) as pcast,
                tc.tile_pool(name="pa_qk", bufs=3, space="PSUM") as pqk,
                tc.tile_pool(name="pa_v", bufs=3, space="PSUM") as ppv,
            ):
                w_sb = paw.tile([128, N_CC, 3 * HPC * HD], BF16)
                for cc in range(N_CC):
                    wst = pcast.tile([128, 3 * HPC * HD], F32, tag="wst")
                    nc.sync.dma_start(out=wst[:], in_=wqkv[cc * 128 : (cc + 1) * 128, :])
                    nc.vector.tensor_copy(w_sb[:, cc, :], wst[:])

                # LN: z tiles to DRAM
                for tb in range(T // TOKB):
                    r0 = tb * TOKB
                    x_t = px.tile([128, HID], F32)
                    nc.sync.dma_start(out=x_t[:], in_=inp[r0 : r0 + 128, :])
                    bn = pst.tile([128, 4, 6], F32, tag="bn")
                    for c4 in range(4):
                        nc.vector.bn_stats(bn[:, c4, :], x_t[:, c4 * 512 : (c4 + 1) * 512])
                    mv = pst.tile([128, 2], F32, tag="mv")
                    nc.vector.bn_aggr(mv[:], bn[:])
                    sd = pst.tile([128, 1], F32, tag="sd")
                    nc.scalar.activation(sd[:], mv[:, 1:2], mybir.ActivationFunctionType.Sqrt, bias=eps_t[:])
                    istd = pst.tile([128, 1], F32, tag="istd")
                    nc.vector.reciprocal(istd[:], sd[:])
                    z_t = pz.tile([128, HID], BF16)
                    nc.vector.tensor_scalar(
                        out=z_t[:],
                        in0=x_t[:],
                        scalar1=mv[:, 0:1],
                        scalar2=istd[:],
                        op0=mybir.AluOpType.subtract,
                        op1=mybir.AluOpType.mult,
                    )
                    nc.sync.dma_start(out=z_dram[tb // N_TB][(tb % N_TB) * TOKB : (tb % N_TB) * TOKB + 128, :], in_=z_t[:])

                # QKV GEMMs per superblock, consuming xbar-transposed z
                for sb in range(N_SB):
                    zT = pzT.tile([128, N_CC, SBK], BF16)
                    for cc in range(N_CC):
                        nc.scalar.dma_start(
                            out=zT[:, cc, :], in_=z_dram[sb][:, cc * 128 : (cc + 1) * 128], transpose=True
                        )
                    for h in range(HPC):
                        for base, bias_col, dst, on_act in (
                            (0, h, qT, True),
                            (HPC * HD, HPC + h, kT, False),
                        ):
                            psq = pqk.tile([128, SBK], F32)
                            for cc in range(N_CC):
                                nc.tensor.matmul(
                                    psq[:],
                                    w_sb[:, cc, base + h * HD : base + (h + 1) * HD],
                                    zT[:, cc, :],
                                    start=(cc == 0),
                                    stop=(cc == N_CC - 1),
                                )
                            dslice = dst[:, h, sb * SBK : (sb + 1) * SBK]
                            if on_act:
                                nc.scalar.activation(
                                    dslice, psq[:], mybir.ActivationFunctionType.Identity,
                                    bias=qkb_sb[:, bias_col : bias_col + 1],
                                )
                            else:
                                nc.vector.tensor_scalar_add(dslice, psq[:], qkb_sb[:, bias_col : bias_col + 1])
                    for tb in range(N_TB):
                        psv = ppv.tile([128, HPC * HD], F32)
                        for cc in range(N_CC):
                            nc.tensor.matmul(
                                psv[:],
                                zT[:, cc, tb * TOKB : tb * TOKB + 128],
                                w_sb[:, cc, 2 * HPC * HD :],
                                start=(cc == 0),
                                stop=(cc == N_CC - 1),
                            )
                        nc.vector.tensor_add(v_sb[:, sb * N_TB + tb, :], psv[:], vb_bc[:])

            # ------------- Phase B/C: attention, A2A, output GEMM -------------
            with (
                tc.tile_pool(name="pb_ow", bufs=1) as pow_,
                tc.tile_pool(name="pb_cast", bufs=2) as pcast2,
                tc.tile_pool(name="pb_p", bufs=6) as pp,
                tc.tile_pool(name="pb_pT", bufs=4) as ppT,
                tc.tile_pool(name="pb_st", bufs=8) as pbs,
                tc.tile_pool(name="pb_rb", bufs=2) as prb,
                tc.tile_pool(name="pb_ctxT", bufs=2) as pcT,
                tc.tile_pool(name="pb_cf", bufs=2) as pcf,
                tc.tile_pool(name="pb_o", bufs=3) as po,
                tc.tile_pool(name="ps_sc", bufs=3, space="PSUM") as pssc,
                tc.tile_pool(name="ps_ct", bufs=2, space="PSUM") as psct,
                tc.tile_pool(name="ps_o", bufs=2, space="PSUM") as pso,
            ):
                ow_sb = pow_.tile([128, N_CC, HID], BF16)
                for cc in range(N_CC):
                    ost = pcast2.tile([128, HID], F32, tag="ost")
                    nc.sync.dma_start(out=ost[:], in_=ow[cc * 128 : (cc + 1) * 128, :])
                    nc.vector.tensor_copy(ow_sb[:, cc, :], ost[:])

                for b in range(B):
                    ctxT = pcT.tile([128, HPC, S], BF16)
                    for h in range(HPC):
                        par = (b * HPC + h) % 2
                        for g in range(N_G):
                            # scores + exp + p-store for the 4 q-blocks of this group
                            recips = pbs.tile([128, N_TB], F32, tag="rcp")
                            for j in range(N_TB):
                                qb = g * N_TB + j
                                span = (qb + 1) * TOKB
                                nkb = (span + SBK - 1) // SBK
                                partials = pbs.tile([128, 4], F32, tag="part")
                                for kb in range(nkb):
                                    w = min(SBK, span - kb * SBK)
                                    ps = pssc.tile([128, SBK], F32)
                                    nc.tensor.matmul(
                                        ps[:, :w],
                                        qT[:, h, b * S + qb * TOKB : b * S + qb * TOKB + 128],
                                        kT[:, h, b * S + kb * SBK : b * S + kb * SBK + w],
                                        start=True,
                                        stop=True,
                                    )
                                    if apply_mask:
                                        nc.vector.tensor_add(
                                            ps[:, :w], ps[:, :w], msk[:, b, kb * SBK : kb * SBK + w]
                                        )
                                    if kb == nkb - 1:
                                        nc.vector.tensor_add(ps[:, w - 128 : w], ps[:, w - 128 : w], causal[:])
                                    p_c = pp.tile([128, SBK], BF16, tag="p")
                                    nc.scalar.activation(
                                        p_c[:, :w],
                                        ps[:, :w],
                                        mybir.ActivationFunctionType.Exp,
                                        scale=SCALE,
                                        accum_out=partials[:, kb : kb + 1],
                                    )
                                    if w < SBK:
                                        nc.vector.memset(p_c[:, w:SBK], 0.0)
                                    nc.sync.dma_start(
                                        out=p_dram[par][g][j * TOKB : (j + 1) * TOKB, kb * SBK : (kb + 1) * SBK],
                                        in_=p_c[:],
                                    )
                                rowsum = pbs.tile([128, 1], F32, tag="rs")
                                nc.vector.tensor_reduce(
                                    rowsum[:], partials[:, 0:nkb], axis=mybir.AxisListType.X, op=mybir.AluOpType.add
                                )
                                nc.vector.reciprocal(recips[:, j : j + 1], rowsum[:])
                            nc.sync.dma_start(
                                out=rs_dram[par][g][:].rearrange("a (j p) -> (a p) j", p=TOKB),
                                in_=recips[:],
                            )
                            # ctx^T for this group
                            rrow = prb.tile([1, SBK], F32, tag="rrow")
                            nc.sync.dma_start(out=rrow[:], in_=rs_dram[par][g][:])
                            rbc = prb.tile([128, SBK], F32, tag="rbc")
                            nc.gpsimd.partition_broadcast(rbc[:], rrow[:])
                            psc = psct.tile([128, SBK], F32)
                            nkc = (g + 1) * N_TB
                            for kc in range(nkc):
                                pT_c = ppT.tile([128, SBK], BF16, tag="pT")
                                nc.scalar.dma_start(
                                    out=pT_c[:], in_=p_dram[par][g][:, kc * 128 : (kc + 1) * 128], transpose=True
                                )
                                nc.tensor.matmul(
                                    psc[:],
                                    v_sb[:, b * (S // 128) + kc, h * HD : (h + 1) * HD],
                                    pT_c[:],
                                    start=(kc == 0),
                                    stop=(kc == nkc - 1),
                                )
                            nc.vector.tensor_tensor(
                                out=ctxT[:, h, g * SBK : (g + 1) * SBK],
                                in0=psc[:],
                                in1=rbc[:],
                                op=mybir.AluOpType.mult,
                            )
                    for j in range(N_CORES):
                        for h in range(HPC):
                            nc.sync.dma_start(
                                out=cc_in[b][j, h * HD : (h + 1) * HD, :],
                                in_=ctxT[:, h, j * TOK_SHARD : (j + 1) * TOK_SHARD],
                            )
                    nc.gpsimd.collective_compute(
                        "AllToAll",
                        mybir.AluOpType.bypass,
                        replica_groups=[list(range(N_CORES))],
                        ins=[cc_in[b][:]],
                        outs=[cc_out[b][:]],
                    )

                # Output GEMM per batch on this core's token shard
                for b in range(B):
                    cf = pcf.tile([128, N_CC, TOK_SHARD], BF16)
                    for cc in range(N_CC):
                        nc.sync.dma_start(
                            out=cf[:, cc, :],
                            in_=cc_out[b][cc // HPC, (cc % HPC) * 128 : (cc % HPC) * 128 + 128, :],
                        )
                    for tb in range(TOK_SHARD // TOKB):
                        for nb in range(HID // 512):
                            pso_t = pso.tile([128, 512], F32)
                            for cc in range(N_CC):
                                nc.tensor.matmul(
                                    pso_t[:],
                                    cf[:, cc, tb * TOKB : tb * TOKB + 128],
                                    ow_sb[:, cc, nb * 512 : (nb + 1) * 512],
                                    start=(cc == 0),
                                    stop=(cc == N_CC - 1),
                                )
                            o_t = po.tile([128, 512], F32)
                            if nb % 2 == 0:
                                nc.scalar.copy(o_t[:], pso_t[:])
                            else:
                                nc.vector.tensor_copy(o_t[:], pso_t[:])
                            nc.sync.dma_start(
                                out=out[
                                    b * TOK_SHARD + tb * TOKB : b * TOK_SHARD + tb * TOKB + 128,
                                    nb * 512 : (nb + 1) * 512,
                                ],
                                in_=o_t[:],
                            )

    nc.compile()
    return nc


_CACHE = {}


def _get_nc(apply_mask: bool):
    if apply_mask not in _CACHE:
        _CACHE[apply_mask] = _build(apply_mask)
    return _CACHE[apply_mask]


def _prep_in_maps(input, input_mask, norm_w, norm_b, attn_qkvw, attn_qkvb, attn_ow):
    x = np.ascontiguousarray(np.asarray(input, dtype=np.float32).reshape(T, HID))
    w = np.asarray(attn_qkvw, dtype=np.float32)
    nw = np.asarray(norm_w, dtype=np.float32)
    nb = np.asarray(norm_b, dtype=np.float32)
    qb_ = np.asarray(attn_qkvb, dtype=np.float32)
    ow = np.ascontiguousarray(np.asarray(attn_ow, dtype=np.float32))
    mask = np.asarray(input_mask, dtype=np.float32).reshape(B, S)

    w_eff = nw[:, None] * w  # fold LN gamma into QKV weight
    b_eff = nb @ w + qb_  # fold LN beta into QKV bias

    apply_mask = bool(np.any(mask != 0.0))
    in_maps = []
    for i in range(N_CORES):
        cols = []
        for part in range(3):  # q, k, v column shards for this core's heads
            c0 = part * HID + i * HPC * HD
            cols.append(w_eff[:, c0 : c0 + HPC * HD])
        wqkv_i = np.ascontiguousarray(np.concatenate(cols, axis=1))

        bq = b_eff[i * HPC * HD : (i + 1) * HPC * HD].reshape(HPC, HD)
        bk = b_eff[HID + i * HPC * HD : HID + (i + 1) * HPC * HD].reshape(HPC, HD)
        qkb_i = np.ascontiguousarray(np.stack([bq[0], bq[1], bk[0], bk[1]], axis=1))  # [128, 4]
        vb_i = np.ascontiguousarray(
            b_eff[2 * HID + i * HPC * HD : 2 * HID + (i + 1) * HPC * HD].reshape(1, HPC * HD)
        )
        m = {"input": x, "qkvw": wqkv_i, "qkb": qkb_i, "vb": vb_i, "ow": ow}
        if apply_mask:
            m["imask"] = np.ascontiguousarray(mask.reshape(1, B * S))
        in_maps.append(m)
    return in_maps, apply_mask


def _run(inputs: dict, trace: bool = False):
    from concourse.bass_utils import run_bass_kernel_spmd

    in_maps, apply_mask = _prep_in_maps(**inputs)
    nc = _get_nc(apply_mask)
    res = run_bass_kernel_spmd(nc, in_maps, list(range(N_CORES)), trace=trace)
    out = np.empty((B, S, HID), dtype=np.float32)
    for j in range(N_CORES):
        o = res.results[j]["out"]
        for b in range(B):
            out[b, j * TOK_SHARD : (j + 1) * TOK_SHARD] = o[b * TOK_SHARD : (b + 1) * TOK_SHARD]
    return out, res


def kernel(**inputs) -> np.ndarray:
    out, _ = _run(inputs, trace=False)
    return out


# revision 11
# speedup vs baseline: 1.7999x; 1.7999x over previous
"""Tensor-parallel DeepSpeed-style self-attention block on 8 TRN2 NeuronCores.

Strategy (head-sharded QKV/attention + all-to-all + token-sharded output GEMM):
  - LayerNorm gamma/beta are folded into the QKV weight/bias on host:
      qkv = z @ (norm_w[:,None]*W) + (norm_b @ W + qkvb),  z=(x-mu)*istd
  - Each core owns 2 of 16 heads: computes z (replicated, bf16), z^T via
    TensorE transposes, then Q^T,K^T (transposed layout) and V (natural
    layout) for its heads over all 4096 tokens.
  - Causal attention per (batch, head): scores = Q^T.T @ K^T in PSUM over
    key-blocks up to the diagonal; triangular -1e9 mask added to the
    diagonal 128x128 block; exp on ScalarE (no max subtraction - scores
    are bounded ~6 sigma for this distribution) with accum_out row-sums.
    p is transposed per 128-block on TensorE into grouped [k,128 x q,512]
    tiles so ctx^T = V.T @ p^T accumulates with N=512 matmuls directly in
    [d, q] (A2A-ready) layout. Row-sum reciprocals ride a tiny DRAM
    bounce to become a [1,512] row, get partition-broadcast on GpSimd,
    and normalize ctx^T on DVE.
  - AllToAll (one per batch, bf16, 1MB) redistributes ctx^T from
    head-sharded to token-sharded: each core ends with all 16 heads for
    its 256-token slice of each batch.
  - Output GEMM: full attn_ow (replicated, cast bf16) x token shard.
    Each core writes a [512, 2048] f32 output shard; host concatenates.
"""

import sys

if "/opt/trn_rl_repo" not in sys.path:
    sys.path.insert(0, "/opt/trn_rl_repo")

# --- shim antenv.axon_hooks (missing in this image) so trace=True can NTFF-profile ---
import types, ctypes, contextlib


def _make_ntff_hook(so_path="/opt/axon/libaxon_pjrt.so"):
    try:
        lib = ctypes.CDLL(so_path)
    except OSError:
        return None
    if not hasattr(lib, "axon_start_nrt_profile"):
        return None
    lib.axon_start_nrt_profile.argtypes = [ctypes.POINTER(ctypes.c_int64), ctypes.c_size_t]
    lib.axon_start_nrt_profile.restype = ctypes.c_int64
    lib.axon_stop_nrt_profile.argtypes = [ctypes.c_char_p]
    lib.axon_stop_nrt_profile.restype = ctypes.c_int64

    @contextlib.contextmanager
    def _hook(output_dir, device_ids):
        import jax

        jax.devices()
        if device_ids:
            ids = (ctypes.c_int64 * len(device_ids))(*device_ids)
            rc = lib.axon_start_nrt_profile(ids, len(device_ids))
        else:
            rc = lib.axon_start_nrt_profile(None, 0)
        if rc != 0:
            raise RuntimeError(f"axon_start_nrt_profile rc={rc}")
        try:
            yield
        finally:
            n = lib.axon_stop_nrt_profile(str(output_dir).encode())
            if n < 0:
                raise RuntimeError(f"axon_stop_nrt_profile rc={n}")

    return _hook


if "antenv.axon_hooks" not in sys.modules:
    _m = types.ModuleType("antenv.axon_hooks")
    _m.get_axon_ntff_profile_hook = lambda: _make_ntff_hook()
    sys.modules["antenv.axon_hooks"] = _m
# --- end shim ---

import numpy as np
import ml_dtypes  # noqa: F401  (bf16 numpy dtype registration)

from concourse import bacc, tile, mybir
from concourse.masks import make_causal_mask, make_identity

B, S, HID = 2, 2048, 2048
HEADS = 16
HD = 128  # head dim
T = B * S  # 4096 tokens
N_CORES = 8
HPC = HEADS // N_CORES  # 2 heads per core
EPS = 1e-6
SCALE = 1.0 / float(np.sqrt(HD))
NEG = -1e9

F32 = mybir.dt.float32
BF16 = mybir.dt.bfloat16

TOKB = 128  # token block (partition dim)
SBK = 512  # superblock of tokens
N_SB = T // SBK  # 8
N_TB = SBK // TOKB  # 4
N_CC = HID // 128  # 16 contraction chunks
N_G = S // SBK  # 4 q-groups of 512 per batch
TOK_SHARD = S // N_CORES  # 256 tokens per (batch, core) after A2A


def _build(apply_mask: bool):
    nc = bacc.Bacc("TRN2", target_bir_lowering=False, debug=False, num_devices=N_CORES)

    inp = nc.dram_tensor("input", [T, HID], F32, kind="ExternalInput").ap()
    wqkv = nc.dram_tensor("qkvw", [HID, 3 * HPC * HD], F32, kind="ExternalInput").ap()
    qkb = nc.dram_tensor("qkb", [128, 2 * HPC], F32, kind="ExternalInput").ap()
    vb = nc.dram_tensor("vb", [1, HPC * HD], F32, kind="ExternalInput").ap()
    ow = nc.dram_tensor("ow", [HID, HID], F32, kind="ExternalInput").ap()
    out = nc.dram_tensor("out", [B * TOK_SHARD, HID], F32, kind="ExternalOutput").ap()
    if apply_mask:
        imask = nc.dram_tensor("imask", [1, B * S], F32, kind="ExternalInput").ap()

    rs_dram = [[nc.dram_tensor(f"rs{par}_{g}", [1, SBK], F32).ap() for g in range(N_G)] for par in range(2)]
    cc_in = [nc.dram_tensor(f"cc_in{b}", [N_CORES, HPC * HD, TOK_SHARD], BF16).ap() for b in range(B)]
    cc_out = [nc.dram_tensor(f"cc_out{b}", [N_CORES, HPC * HD, TOK_SHARD], BF16).ap() for b in range(B)]

    with tile.TileContext(nc) as tc:
        with tc.tile_pool(name="persist", bufs=1) as pers:
            ident = pers.tile([128, 128], BF16)
            make_identity(nc, ident[:])
            causal = pers.tile([128, 128], F32)
            make_causal_mask(nc, causal[:], mask_val=NEG)
            eps_t = pers.tile([128, 1], F32)
            nc.gpsimd.memset(eps_t[:], EPS)
            qkb_sb = pers.tile([128, 2 * HPC], F32)
            nc.sync.dma_start(out=qkb_sb[:], in_=qkb[:])
            vbf = pers.tile([1, HPC * HD], F32)
            nc.sync.dma_start(out=vbf[:], in_=vb[:])
            vb_bc = pers.tile([128, HPC * HD], F32)
            nc.gpsimd.partition_broadcast(vb_bc[:], vbf[:])

            qT = pers.tile([128, HPC, T], BF16)  # [d, head, tok]
            kT = pers.tile([128, HPC, T], BF16)
            v_sb = pers.tile([128, T // 128, HPC * HD], BF16)  # [tok128, blk, hcol]

            if apply_mask:
                msk = pers.tile([128, B, S], F32)
                mrow = pers.tile([1, B * S], F32)
                nc.sync.dma_start(out=mrow[:], in_=imask[:])
                for b in range(B):
                    nc.gpsimd.partition_broadcast(msk[:, b, :], mrow[:, b * S : (b + 1) * S])

            # ---------------- Phase A: LN + z^T (TensorE) + QKV GEMM ----------------
            with (
                tc.tile_pool(name="pa_w", bufs=1) as paw,
                tc.tile_pool(name="pa_x", bufs=3) as px,
                tc.tile_pool(name="pa_st", bufs=6) as pst,
                tc.tile_pool(name="pa_z", bufs=3) as pz,
                tc.tile_pool(name="pa_zT", bufs=2) as pzT,
                tc.tile_pool(name="pa_cast", bufs=2) as pcast,
                tc.tile_pool(name="pa_tr", bufs=3, space="PSUM") as ptrA,
                tc.tile_pool(name="pa_qk", bufs=3, space="PSUM") as pqk,
                tc.tile_pool(name="pa_v", bufs=2, space="PSUM") as ppv,
            ):
                w_sb = paw.tile([128, N_CC, 3 * HPC * HD], BF16)
                for cc in range(N_CC):
                    wst = pcast.tile([128, 3 * HPC * HD], F32, tag="wst")
                    nc.sync.dma_start(out=wst[:], in_=wqkv[cc * 128 : (cc + 1) * 128, :])
                    nc.vector.tensor_copy(w_sb[:, cc, :], wst[:])

                for sb in range(N_SB):
                    zT = pzT.tile([128, N_CC, SBK], BF16)
                    for tb in range(N_TB):
                        r0 = sb * SBK + tb * TOKB
                        x_t = px.tile([128, HID], F32)
                        nc.sync.dma_start(out=x_t[:], in_=inp[r0 : r0 + 128, :])
                        bn = pst.tile([128, 4, 6], F32, tag="bn")
                        for c4 in range(4):
                            nc.vector.bn_stats(bn[:, c4, :], x_t[:, c4 * 512 : (c4 + 1) * 512])
                        mv = pst.tile([128, 2], F32, tag="mv")
                        nc.vector.bn_aggr(mv[:], bn[:])
                        sd = pst.tile([128, 1], F32, tag="sd")
                        nc.scalar.activation(sd[:], mv[:, 1:2], mybir.ActivationFunctionType.Sqrt, bias=eps_t[:])
                        istd = pst.tile([128, 1], F32, tag="istd")
                        nc.vector.reciprocal(istd[:], sd[:])
                        z_t = pz.tile([128, HID], BF16)
                        nc.vector.tensor_scalar(
                            out=z_t[:],
                            in0=x_t[:],
                            scalar1=mv[:, 0:1],
                            scalar2=istd[:],
                            op0=mybir.AluOpType.subtract,
                            op1=mybir.AluOpType.mult,
                        )
                        for cc in range(N_CC):
                            ps_t = ptrA.tile([128, 128], BF16)
                            nc.tensor.transpose(ps_t[:], z_t[:, cc * 128 : (cc + 1) * 128], ident[:])
                            if cc % 2 == 0:
                                nc.scalar.copy(zT[:, cc, tb * TOKB : tb * TOKB + 128], ps_t[:])
                            else:
                                nc.vector.tensor_copy(zT[:, cc, tb * TOKB : tb * TOKB + 128], ps_t[:])

                    for h in range(HPC):
                        for base, bias_col, dst, on_act in (
                            (0, h, qT, True),
                            (HPC * HD, HPC + h, kT, False),
                        ):
                            psq = pqk.tile([128, SBK], F32)
                            for cc in range(N_CC):
                                nc.tensor.matmul(
                                    psq[:],
                                    w_sb[:, cc, base + h * HD : base + (h + 1) * HD],
                                    zT[:, cc, :],
                                    start=(cc == 0),
                                    stop=(cc == N_CC - 1),
                                )
                            dslice = dst[:, h, sb * SBK : (sb + 1) * SBK]
                            if on_act:
                                nc.scalar.activation(
                                    dslice, psq[:], mybir.ActivationFunctionType.Identity,
                                    bias=qkb_sb[:, bias_col : bias_col + 1],
                                )
                            else:
                                nc.vector.tensor_scalar_add(dslice, psq[:], qkb_sb[:, bias_col : bias_col + 1])
                    for tb in range(N_TB):
                        psv = ppv.tile([128, HPC * HD], F32)
                        for cc in range(N_CC):
                            nc.tensor.matmul(
                                psv[:],
                                zT[:, cc, tb * TOKB : tb * TOKB + 128],
                                w_sb[:, cc, 2 * HPC * HD :],
                                start=(cc == 0),
                                stop=(cc == N_CC - 1),
                            )
                        nc.vector.tensor_add(v_sb[:, sb * N_TB + tb, :], psv[:], vb_bc[:])

            # ------------- Phase B/C: attention, A2A, output GEMM -------------
            with (
                tc.tile_pool(name="pb_ow", bufs=1) as pow_,
                tc.tile_pool(name="pb_cast", bufs=1) as pcast2,
                tc.tile_pool(name="pb_p", bufs=5) as pp,
                tc.tile_pool(name="pb_pT", bufs=6) as ppT,
                tc.tile_pool(name="pb_st", bufs=8) as pbs,
                tc.tile_pool(name="pb_rb", bufs=2) as prb,
                tc.tile_pool(name="pb_ctxT", bufs=2) as pcT,
                tc.tile_pool(name="pb_cf", bufs=2) as pcf,
                tc.tile_pool(name="pb_o", bufs=2) as po,
                tc.tile_pool(name="ps_sc", bufs=2, space="PSUM") as pssc,
                tc.tile_pool(name="ps_tr", bufs=2, space="PSUM") as pstr,
                tc.tile_pool(name="ps_ct", bufs=2, space="PSUM") as psct,
                tc.tile_pool(name="ps_o", bufs=2, space="PSUM") as pso,
            ):
                ow_sb = pow_.tile([128, N_CC, HID], BF16)
                for cc in range(N_CC):
                    ost = pcast2.tile([128, HID], F32, tag="ost")
                    nc.sync.dma_start(out=ost[:], in_=ow[cc * 128 : (cc + 1) * 128, :])
                    nc.vector.tensor_copy(ow_sb[:, cc, :], ost[:])

                for b in range(B):
                    ctxT = pcT.tile([128, HPC, S], BF16)
                    for h in range(HPC):
                        par = (b * HPC + h) % 2
                        for g in range(N_G):
                            recips = pbs.tile([128, N_TB], F32, tag="rcp")
                            p_tiles = []
                            for j in range(N_TB):
                                qb = g * N_TB + j
                                span = (qb + 1) * TOKB
                                nkb = (span + SBK - 1) // SBK
                                p_t = pp.tile([128, S], BF16, tag="p")
                                partials = pbs.tile([128, 4], F32, tag="part")
                                for kb in range(nkb):
                                    w = min(SBK, span - kb * SBK)
                                    ps = pssc.tile([128, SBK], F32)
                                    nc.tensor.matmul(
                                        ps[:, :w],
                                        qT[:, h, b * S + qb * TOKB : b * S + qb * TOKB + 128],
                                        kT[:, h, b * S + kb * SBK : b * S + kb * SBK + w],
                                        start=True,
                                        stop=True,
                                    )
                                    if apply_mask:
                                        nc.vector.tensor_add(
                                            ps[:, :w], ps[:, :w], msk[:, b, kb * SBK : kb * SBK + w]
                                        )
                                    if kb == nkb - 1:
                                        nc.vector.tensor_add(ps[:, w - 128 : w], ps[:, w - 128 : w], causal[:])
                                    nc.scalar.activation(
                                        p_t[:, kb * SBK : kb * SBK + w],
                                        ps[:, :w],
                                        mybir.ActivationFunctionType.Exp,
                                        scale=SCALE,
                                        accum_out=partials[:, kb : kb + 1],
                                    )
                                p_tiles.append(p_t)
                                rowsum = pbs.tile([128, 1], F32, tag="rs")
                                nc.vector.tensor_reduce(
                                    rowsum[:], partials[:, 0:nkb], axis=mybir.AxisListType.X, op=mybir.AluOpType.add
                                )
                                nc.vector.reciprocal(recips[:, j : j + 1], rowsum[:])
                            nc.sync.dma_start(
                                out=rs_dram[par][g][:].rearrange("a (j p) -> (a p) j", p=TOKB),
                                in_=recips[:],
                            )
                            rrow = prb.tile([1, SBK], F32, tag="rrow")
                            nc.sync.dma_start(out=rrow[:], in_=rs_dram[par][g][:])
                            rbc = prb.tile([128, SBK], F32, tag="rbc")
                            nc.gpsimd.partition_broadcast(rbc[:], rrow[:])

                            psc = psct.tile([128, SBK], F32)
                            nkc = (g + 1) * N_TB
                            for kc in range(nkc):
                                pT_g = ppT.tile([128, SBK], BF16, tag="pT")
                                d0 = kc - 4 * g  # first valid q-slot on the diagonal
                                if d0 > 0:
                                    nc.vector.memset(pT_g[:, : d0 * TOKB], 0.0)
                                for j in range(max(0, d0), N_TB):
                                    ps_t = pstr.tile([128, 128], BF16)
                                    nc.tensor.transpose(
                                        ps_t[:], p_tiles[j][:, kc * 128 : (kc + 1) * 128], ident[:]
                                    )
                                    if (kc + j) % 2 == 0:
                                        nc.scalar.copy(pT_g[:, j * TOKB : (j + 1) * TOKB], ps_t[:])
                                    else:
                                        nc.vector.tensor_copy(pT_g[:, j * TOKB : (j + 1) * TOKB], ps_t[:])
                                nc.tensor.matmul(
                                    psc[:],
                                    v_sb[:, b * (S // 128) + kc, h * HD : (h + 1) * HD],
                                    pT_g[:],
                                    start=(kc == 0),
                                    stop=(kc == nkc - 1),
                                )
                            nc.vector.tensor_tensor(
                                out=ctxT[:, h, g * SBK : (g + 1) * SBK],
                                in0=psc[:],
                                in1=rbc[:],
                                op=mybir.AluOpType.mult,
                            )
                    for j in range(N_CORES):
                        for h in range(HPC):
                            nc.sync.dma_start(
                                out=cc_in[b][j, h * HD : (h + 1) * HD, :],
                                in_=ctxT[:, h, j * TOK_SHARD : (j + 1) * TOK_SHARD],
                            )
                    nc.gpsimd.collective_compute(
                        "AllToAll",
                        mybir.AluOpType.bypass,
                        replica_groups=[list(range(N_CORES))],
                        ins=[cc_in[b][:]],
                        outs=[cc_out[b][:]],
                    )

                # Output GEMM per batch on this core's token shard
                for b in range(B):
                    cf = pcf.tile([128, N_CC, TOK_SHARD], BF16)
                    for cc in range(N_CC):
                        nc.sync.dma_start(
                            out=cf[:, cc, :],
                            in_=cc_out[b][cc // HPC, (cc % HPC) * 128 : (cc % HPC) * 128 + 128, :],
                        )
                    for tb in range(TOK_SHARD // TOKB):
                        for nb in range(HID // 512):
                            pso_t = pso.tile([128, 512], F32)
                            for cc in range(N_CC):
                                nc.tensor.matmul(
                                    pso_t[:],
                                    cf[:, cc, tb * TOKB : tb * TOKB + 128],
                                    ow_sb[:, cc, nb * 512 : (nb + 1) * 512],
                                    start=(cc == 0),
                                    stop=(cc == N_CC - 1),
                                )
                            o_t = po.tile([128, 512], F32)
                            if nb % 2 == 0:
                                nc.scalar.copy(o_t[:], pso_t[:])
                            else:
                                nc.vector.tensor_copy(o_t[:], pso_t[:])
                            nc.sync.dma_start(
                                out=out[
                                    b * TOK_SHARD + tb * TOKB : b * TOK_SHARD + tb * TOKB + 128,
                                    nb * 512 : (nb + 1) * 512,
                                ],
                                in_=o_t[:],
                            )

    nc.compile()
    return nc


_CACHE = {}


def _get_nc(apply_mask: bool):
    if apply_mask not in _CACHE:
        _CACHE[apply_mask] = _build(apply_mask)
    return _CACHE[apply_mask]


def _prep_in_maps(input, input_mask, norm_w, norm_b, attn_qkvw, attn_qkvb, attn_ow):
    x = np.ascontiguousarray(np.asarray(input, dtype=np.float32).reshape(T, HID))
    w = np.asarray(attn_qkvw, dtype=np.float32)
    nw = np.asarray(norm_w, dtype=np.float32)
    nb = np.asarray(norm_b, dtype=np.float32)
    qb_ = np.asarray(attn_qkvb, dtype=np.float32)
    ow = np.ascontiguousarray(np.asarray(attn_ow, dtype=np.float32))
    mask = np.asarray(input_mask, dtype=np.float32).reshape(B, S)

    w_eff = nw[:, None] * w  # fold LN gamma into QKV weight
    b_eff = nb @ w + qb_  # fold LN beta into QKV bias

    apply_mask = bool(np.any(mask != 0.0))
    in_maps = []
    for i in range(N_CORES):
        cols = []
        for part in range(3):  # q, k, v column shards for this core's heads
            c0 = part * HID + i * HPC * HD
            cols.append(w_eff[:, c0 : c0 + HPC * HD])
        wqkv_i = np.ascontiguousarray(np.concatenate(cols, axis=1))

        bq = b_eff[i * HPC * HD : (i + 1) * HPC * HD].reshape(HPC, HD)
        bk = b_eff[HID + i * HPC * HD : HID + (i + 1) * HPC * HD].reshape(HPC, HD)
        qkb_i = np.ascontiguousarray(np.stack([bq[0], bq[1], bk[0], bk[1]], axis=1))  # [128, 4]
        vb_i = np.ascontiguousarray(
            b_eff[2 * HID + i * HPC * HD : 2 * HID + (i + 1) * HPC * HD].reshape(1, HPC * HD)
        )
        m = {"input": x, "qkvw": wqkv_i, "qkb": qkb_i, "vb": vb_i, "ow": ow}
        if apply_mask:
            m["imask"] = np.ascontiguousarray(mask.reshape(1, B * S))
        in_maps.append(m)
    return in_maps, apply_mask


def _run(inputs: dict, trace: bool = False):
    from concourse.bass_utils import run_bass_kernel_spmd

    in_maps, apply_mask = _prep_in_maps(**inputs)
    nc = _get_nc(apply_mask)
    res = run_bass_kernel_spmd(nc, in_maps, list(range(N_CORES)), trace=trace)
    out = np.empty((B, S, HID), dtype=np.float32)
    for j in range(N_CORES):
        o = res.results[j]["out"]
        for b in range(B):
            out[b, j * TOK_SHARD : (j + 1) * TOK_SHARD] = o[b * TOK_SHARD : (b + 1) * TOK_SHARD]
    return out, res


def kernel(**inputs) -> np.ndarray:
    out, _ = _run(inputs, trace=False)
    return out


# revision 13
# speedup vs baseline: 1.8830x; 1.0462x over previous
"""Tensor-parallel DeepSpeed-style self-attention block on 8 TRN2 NeuronCores.

Strategy (head-sharded QKV/attention + all-to-all + token-sharded output GEMM):
  - LayerNorm params are folded into the QKV weight/bias on host:
      qkv = z @ (norm_w[:,None]*W) + (norm_b @ W + qkvb),  z=(x-mu)*istd
  - Each core owns 2 of 16 heads: computes z (replicated), z^T via PE
    transposes, then Q^T,K^T (transposed layout) and V (natural layout)
    for its heads over all 4096 tokens.
  - Causal attention per (batch, head): scores = Q^T.T @ K^T in PSUM,
    only key-blocks <= diagonal; triangular mask added to the diagonal
    128x128 block; exp on ScalarE with accum_out row-sums (no max
    subtraction: |scores/sqrt(d)| <= ~6 for this distribution);
    p transposed per 128-chunk on PE; ctx = p^T.T @ V accumulated in
    PSUM; normalized by 1/rowsum during PSUM->SBUF copy; transposed to
    ctx^T.
  - AllToAll (one per batch, bf16, 1MB) redistributes ctx^T from
    head-sharded to token-sharded: each core ends with all 16 heads for
    its 256-token slice of each batch.
  - Output GEMM: full attn_ow (replicated, cast bf16) x token shard.
    Each core writes a [512, 2048] f32 output shard; host concatenates.
"""

import sys

if "/opt/trn_rl_repo" not in sys.path:
    sys.path.insert(0, "/opt/trn_rl_repo")

# --- shim antenv.axon_hooks (missing in this image) so trace=True can NTFF-profile ---
import types, ctypes, contextlib


def _make_ntff_hook(so_path="/opt/axon/libaxon_pjrt.so"):
    try:
        lib = ctypes.CDLL(so_path)
    except OSError:
        return None
    if not hasattr(lib, "axon_start_nrt_profile"):
        return None
    lib.axon_start_nrt_profile.argtypes = [ctypes.POINTER(ctypes.c_int64), ctypes.c_size_t]
    lib.axon_start_nrt_profile.restype = ctypes.c_int64
    lib.axon_stop_nrt_profile.argtypes = [ctypes.c_char_p]
    lib.axon_stop_nrt_profile.restype = ctypes.c_int64

    @contextlib.contextmanager
    def _hook(output_dir, device_ids):
        import jax

        jax.devices()
        if device_ids:
            ids = (ctypes.c_int64 * len(device_ids))(*device_ids)
            rc = lib.axon_start_nrt_profile(ids, len(device_ids))
        else:
            rc = lib.axon_start_nrt_profile(None, 0)
        if rc != 0:
            raise RuntimeError(f"axon_start_nrt_profile rc={rc}")
        try:
            yield
        finally:
            n = lib.axon_stop_nrt_profile(str(output_dir).encode())
            if n < 0:
                raise RuntimeError(f"axon_stop_nrt_profile rc={n}")

    return _hook


if "antenv.axon_hooks" not in sys.modules:
    _m = types.ModuleType("antenv.axon_hooks")
    _m.get_axon_ntff_profile_hook = lambda: _make_ntff_hook()
    sys.modules["antenv.axon_hooks"] = _m
# --- end shim ---

import numpy as np
import ml_dtypes  # noqa: F401  (bf16 numpy dtype registration)

from concourse import bacc, tile, mybir
from concourse.masks import make_causal_mask, make_identity

B, S, HID = 2, 2048, 2048
HEADS = 16
HD = 128  # head dim
T = B * S  # 4096 tokens
N_CORES = 8
HPC = HEADS // N_CORES  # 2 heads per core
EPS = 1e-6
SCALE = 1.0 / float(np.sqrt(HD))
NEG = -1e9

F32 = mybir.dt.float32
BF16 = mybir.dt.bfloat16

TOKB = 128  # token block (partition dim)
SB = 512  # superblock of tokens for QKV GEMM
N_SB = T // SB  # 8
N_TB = SB // TOKB  # 4
N_CC = HID // 128  # 16 contraction chunks
TOK_SHARD = S // N_CORES  # 256 tokens per (batch, core) after A2A


def _build(apply_mask: bool):
    nc = bacc.Bacc("TRN2", target_bir_lowering=False, debug=False, num_devices=N_CORES)

    inp = nc.dram_tensor("input", [T, HID], F32, kind="ExternalInput").ap()
    wqkv = nc.dram_tensor("qkvw", [HID, 3 * HPC * HD], F32, kind="ExternalInput").ap()
    qkb = nc.dram_tensor("qkb", [128, 2 * HPC], F32, kind="ExternalInput").ap()
    vb = nc.dram_tensor("vb", [1, HPC * HD], F32, kind="ExternalInput").ap()
    ow = nc.dram_tensor("ow", [HID, HID], F32, kind="ExternalInput").ap()
    out = nc.dram_tensor("out", [B * TOK_SHARD, HID], F32, kind="ExternalOutput").ap()
    if apply_mask:
        imask = nc.dram_tensor("imask", [1, B * S], F32, kind="ExternalInput").ap()

    cc_in = [nc.dram_tensor(f"cc_in{b}", [N_CORES, HPC * HD, TOK_SHARD], BF16).ap() for b in range(B)]
    cc_out = [nc.dram_tensor(f"cc_out{b}", [N_CORES, HPC * HD, TOK_SHARD], BF16).ap() for b in range(B)]

    with tile.TileContext(nc) as tc:
        with tc.tile_pool(name="persist", bufs=1) as pers:
            ident = pers.tile([128, 128], BF16)
            make_identity(nc, ident[:])
            causal = pers.tile([128, 128], F32)
            make_causal_mask(nc, causal[:], mask_val=NEG)
            eps_t = pers.tile([128, 1], F32)
            nc.gpsimd.memset(eps_t[:], EPS)
            qkb_sb = pers.tile([128, 2 * HPC], F32)
            nc.sync.dma_start(out=qkb_sb[:], in_=qkb[:])
            vbf = pers.tile([1, HPC * HD], F32)
            nc.sync.dma_start(out=vbf[:], in_=vb[:])
            vb_bc = pers.tile([128, HPC * HD], F32)
            nc.gpsimd.partition_broadcast(vb_bc[:], vbf[:])

            qT = pers.tile([128, HPC, T], BF16)  # [d, head, tok]
            kT = pers.tile([128, HPC, T], BF16)
            v_sb = pers.tile([128, T // 128, HPC * HD], BF16)  # [tok128, blk, hcol]

            if apply_mask:
                msk = pers.tile([128, B, S], F32)
                mrow = pers.tile([1, B * S], F32)
                nc.sync.dma_start(out=mrow[:], in_=imask[:])
                for b in range(B):
                    nc.gpsimd.partition_broadcast(msk[:, b, :], mrow[:, b * S : (b + 1) * S])

            # ---------------- Phase A: LN + z^T + QKV GEMM ----------------
            with (
                tc.tile_pool(name="pa_w", bufs=1) as paw,
                tc.tile_pool(name="pa_x", bufs=3) as px,
                tc.tile_pool(name="pa_st", bufs=6) as pst,
                tc.tile_pool(name="pa_z", bufs=3) as pz,
                tc.tile_pool(name="pa_zT", bufs=2) as pzT,
                tc.tile_pool(name="pa_cast", bufs=2) as pcast,
                tc.tile_pool(name="pa_tr", bufs=3, space="PSUM") as ptr,
                tc.tile_pool(name="pa_qk", bufs=3, space="PSUM") as pqk,
                tc.tile_pool(name="pa_v", bufs=2, space="PSUM") as ppv,
            ):
                w_sb = paw.tile([128, N_CC, 3 * HPC * HD], BF16)
                for cc in range(N_CC):
                    wst = pcast.tile([128, 3 * HPC * HD], F32, tag="wst")
                    nc.sync.dma_start(out=wst[:], in_=wqkv[cc * 128 : (cc + 1) * 128, :])
                    nc.vector.tensor_copy(w_sb[:, cc, :], wst[:])

                for sb in range(N_SB):
                    zT = pzT.tile([128, N_CC, SB], BF16)
                    for tb in range(N_TB):
                        r0 = sb * SB + tb * TOKB
                        x_t = px.tile([128, HID], F32)
                        nc.sync.dma_start(out=x_t[:], in_=inp[r0 : r0 + 128, :])
                        bn = pst.tile([128, 4, 6], F32, tag="bn")
                        for c4 in range(4):
                            nc.vector.bn_stats(bn[:, c4, :], x_t[:, c4 * 512 : (c4 + 1) * 512])
                        mv = pst.tile([128, 2], F32, tag="mv")
                        nc.vector.bn_aggr(mv[:], bn[:])
                        sd = pst.tile([128, 1], F32, tag="sd")
                        nc.scalar.activation(sd[:], mv[:, 1:2], mybir.ActivationFunctionType.Sqrt, bias=eps_t[:])
                        istd = pst.tile([128, 1], F32, tag="istd")
                        nc.vector.reciprocal(istd[:], sd[:])
                        z_t = pz.tile([128, HID], BF16)
                        nc.vector.tensor_scalar(
                            out=z_t[:],
                            in0=x_t[:],
                            scalar1=mv[:, 0:1],
                            scalar2=istd[:],
                            op0=mybir.AluOpType.subtract,
                            op1=mybir.AluOpType.mult,
                        )
                        for cc in range(N_CC):
                            ps_t = ptr.tile([128, 128], BF16)
                            nc.tensor.transpose(ps_t[:], z_t[:, cc * 128 : (cc + 1) * 128], ident[:])
                            if cc % 2 == 0:
                                nc.scalar.copy(zT[:, cc, tb * TOKB : tb * TOKB + 128], ps_t[:])
                            else:
                                nc.vector.tensor_copy(zT[:, cc, tb * TOKB : tb * TOKB + 128], ps_t[:])

                    # Q^T, K^T for this superblock (transposed GEMM)
                    for h in range(HPC):
                        for base, bias_col, dst, on_act in (
                            (0, h, qT, True),
                            (HPC * HD, HPC + h, kT, False),
                        ):
                            psq = pqk.tile([128, SB], F32)
                            for cc in range(N_CC):
                                nc.tensor.matmul(
                                    psq[:],
                                    w_sb[:, cc, base + h * HD : base + (h + 1) * HD],
                                    zT[:, cc, :],
                                    start=(cc == 0),
                                    stop=(cc == N_CC - 1),
                                )
                            dslice = dst[:, h, sb * SB : (sb + 1) * SB]
                            if on_act:
                                nc.scalar.activation(
                                    dslice, psq[:], mybir.ActivationFunctionType.Identity,
                                    bias=qkb_sb[:, bias_col : bias_col + 1],
                                )
                            else:
                                nc.vector.tensor_scalar_add(dslice, psq[:], qkb_sb[:, bias_col : bias_col + 1])
                    # V natural
                    for tb in range(N_TB):
                        psv = ppv.tile([128, HPC * HD], F32)
                        for cc in range(N_CC):
                            nc.tensor.matmul(
                                psv[:],
                                zT[:, cc, tb * TOKB : tb * TOKB + 128],
                                w_sb[:, cc, 2 * HPC * HD :],
                                start=(cc == 0),
                                stop=(cc == N_CC - 1),
                            )
                        nc.vector.tensor_add(v_sb[:, sb * N_TB + tb, :], psv[:], vb_bc[:])

            # ------------- Phase B/C: attention, A2A, output GEMM -------------
            with (
                tc.tile_pool(name="pb_ow", bufs=1) as pow_,
                tc.tile_pool(name="pb_cast", bufs=2) as pcast2,
                tc.tile_pool(name="pb_p", bufs=6) as pp,
                tc.tile_pool(name="pb_pT", bufs=6) as ppT,
                tc.tile_pool(name="pb_st", bufs=8) as pbs,
                tc.tile_pool(name="pb_ctx", bufs=3) as pctx,
                tc.tile_pool(name="pb_ctxT", bufs=2) as pcT,
                tc.tile_pool(name="pb_cf", bufs=2) as pcf,
                tc.tile_pool(name="pb_o", bufs=3) as po,
                tc.tile_pool(name="ps_sc", bufs=2, space="PSUM") as pssc,
                tc.tile_pool(name="ps_tr", bufs=2, space="PSUM") as pstr,
                tc.tile_pool(name="ps_ctx", bufs=1, space="PSUM") as psctx,
                tc.tile_pool(name="ps_o", bufs=2, space="PSUM") as pso,
            ):
                ow_sb = pow_.tile([128, N_CC, HID], BF16)
                for cc in range(N_CC):
                    ost = pcast2.tile([128, HID], F32, tag="ost")
                    nc.sync.dma_start(out=ost[:], in_=ow[cc * 128 : (cc + 1) * 128, :])
                    nc.vector.tensor_copy(ow_sb[:, cc, :], ost[:])

                for b in range(B):
                    ctxT = pcT.tile([128, HPC, S], BF16)
                    for h in range(HPC):
                        for qb in range(S // TOKB):
                            span = (qb + 1) * TOKB
                            nkb = (span + 511) // 512
                            p_chunks = []
                            partials = pbs.tile([128, 4], F32, tag="part")
                            for kb in range(nkb):
                                w = min(512, span - kb * 512)
                                ps = pssc.tile([128, 512], F32)
                                nc.tensor.matmul(
                                    ps[:, :w],
                                    qT[:, h, b * S + qb * TOKB : b * S + qb * TOKB + 128],
                                    kT[:, h, b * S + kb * 512 : b * S + kb * 512 + w],
                                    start=True,
                                    stop=True,
                                )
                                if apply_mask:
                                    nc.vector.tensor_add(
                                        ps[:, :w], ps[:, :w], msk[:, b, kb * 512 : kb * 512 + w]
                                    )
                                if kb == nkb - 1:
                                    nc.vector.tensor_add(ps[:, w - 128 : w], ps[:, w - 128 : w], causal[:])
                                p_c = pp.tile([128, 512], BF16, tag="p")
                                nc.scalar.activation(
                                    p_c[:, :w],
                                    ps[:, :w],
                                    mybir.ActivationFunctionType.Exp,
                                    scale=SCALE,
                                    accum_out=partials[:, kb : kb + 1],
                                )
                                p_chunks.append(p_c)
                            rowsum = pbs.tile([128, 1], F32, tag="rs")
                            nc.vector.tensor_reduce(
                                rowsum[:], partials[:, 0:nkb], axis=mybir.AxisListType.X, op=mybir.AluOpType.add
                            )
                            recip = pbs.tile([128, 1], F32, tag="rc")
                            nc.vector.reciprocal(recip[:], rowsum[:])

                            psc = psctx.tile([128, HD], F32)
                            nkc = qb + 1
                            for kc in range(nkc):
                                pt_ps = pstr.tile([128, 128], BF16, tag="trp")
                                nc.tensor.transpose(
                                    pt_ps[:], p_chunks[kc // 4][:, (kc % 4) * 128 : (kc % 4) * 128 + 128], ident[:]
                                )
                                pT_c = ppT.tile([128, 128], BF16, tag="pT")
                                if kc % 2 == 0:
                                    nc.scalar.copy(pT_c[:], pt_ps[:])
                                else:
                                    nc.vector.tensor_copy(pT_c[:], pt_ps[:])
                                nc.tensor.matmul(
                                    psc[:],
                                    pT_c[:],
                                    v_sb[:, b * (S // 128) + kc, h * HD : (h + 1) * HD],
                                    start=(kc == 0),
                                    stop=(kc == nkc - 1),
                                )
                            ctx_t = pctx.tile([128, HD], BF16)
                            nc.scalar.mul(ctx_t[:], psc[:], recip[:])
                            ct_ps = pstr.tile([128, 128], BF16, tag="trp")
                            nc.tensor.transpose(ct_ps[:], ctx_t[:], ident[:])
                            nc.vector.tensor_copy(ctxT[:, h, qb * TOKB : qb * TOKB + 128], ct_ps[:])
                    for j in range(N_CORES):
                        for h in range(HPC):
                            nc.sync.dma_start(
                                out=cc_in[b][j, h * HD : (h + 1) * HD, :],
                                in_=ctxT[:, h, j * TOK_SHARD : (j + 1) * TOK_SHARD],
                            )
                    nc.gpsimd.collective_compute(
                        "AllToAll",
                        mybir.AluOpType.bypass,
                        replica_groups=[list(range(N_CORES))],
                        ins=[cc_in[b][:]],
                        outs=[cc_out[b][:]],
                    )

                # Output GEMM per batch on this core's token shard
                for b in range(B):
                    cf = pcf.tile([128, N_CC, TOK_SHARD], BF16)
                    for cc in range(N_CC):
                        nc.sync.dma_start(
                            out=cf[:, cc, :],
                            in_=cc_out[b][cc // HPC, (cc % HPC) * 128 : (cc % HPC) * 128 + 128, :],
                        )
                    for tb in range(TOK_SHARD // TOKB):
                        for nb in range(HID // 512):
                            pso_t = pso.tile([128, 512], F32)
                            for cc in range(N_CC):
                                nc.tensor.matmul(
                                    pso_t[:],
                                    cf[:, cc, tb * TOKB : tb * TOKB + 128],
                                    ow_sb[:, cc, nb * 512 : (nb + 1) * 512],
                                    start=(cc == 0),
                                    stop=(cc == N_CC - 1),
                                )
                            o_t = po.tile([128, 512], F32)
                            if nb % 2 == 0:
                                nc.scalar.copy(o_t[:], pso_t[:])
                            else:
                                nc.vector.tensor_copy(o_t[:], pso_t[:])
                            nc.sync.dma_start(
                                out=out[b * TOK_SHARD + tb * TOKB : b * TOK_SHARD + tb * TOKB + 128,
                                        nb * 512 : (nb + 1) * 512],
                                in_=o_t[:],
                            )

    nc.compile()
    return nc


_CACHE = {}


def _get_nc(apply_mask: bool):
    if apply_mask not in _CACHE:
        _CACHE[apply_mask] = _build(apply_mask)
    return _CACHE[apply_mask]


def _prep_in_maps(input, input_mask, norm_w, norm_b, attn_qkvw, attn_qkvb, attn_ow):
    x = np.ascontiguousarray(np.asarray(input, dtype=np.float32).reshape(T, HID))
    w = np.asarray(attn_qkvw, dtype=np.float32)
    nw = np.asarray(norm_w, dtype=np.float32)
    nb = np.asarray(norm_b, dtype=np.float32)
    qb_ = np.asarray(attn_qkvb, dtype=np.float32)
    ow = np.ascontiguousarray(np.asarray(attn_ow, dtype=np.float32))
    mask = np.asarray(input_mask, dtype=np.float32).reshape(B, S)

    w_eff = nw[:, None] * w  # fold LN gamma into QKV weight
    b_eff = nb @ w + qb_  # fold LN beta into QKV bias

    apply_mask = bool(np.any(mask != 0.0))
    in_maps = []
    for i in range(N_CORES):
        cols = []
        for part in range(3):  # q, k, v column shards for this core's heads
            c0 = part * HID + i * HPC * HD
            cols.append(w_eff[:, c0 : c0 + HPC * HD])
        wqkv_i = np.ascontiguousarray(np.concatenate(cols, axis=1))

        bq = b_eff[i * HPC * HD : (i + 1) * HPC * HD].reshape(HPC, HD)
        bk = b_eff[HID + i * HPC * HD : HID + (i + 1) * HPC * HD].reshape(HPC, HD)
        qkb_i = np.ascontiguousarray(np.stack([bq[0], bq[1], bk[0], bk[1]], axis=1))  # [128, 4]
        vb_i = np.ascontiguousarray(
            b_eff[2 * HID + i * HPC * HD : 2 * HID + (i + 1) * HPC * HD].reshape(1, HPC * HD)
        )
        m = {"input": x, "qkvw": wqkv_i, "qkb": qkb_i, "vb": vb_i, "ow": ow}
        if apply_mask:
            m["imask"] = np.ascontiguousarray(mask.reshape(1, B * S))
        in_maps.append(m)
    return in_maps, apply_mask


def _run(inputs: dict, trace: bool = False):
    from concourse.bass_utils import run_bass_kernel_spmd

    in_maps, apply_mask = _prep_in_maps(**inputs)
    nc = _get_nc(apply_mask)
    res = run_bass_kernel_spmd(nc, in_maps, list(range(N_CORES)), trace=trace)
    out = np.empty((B, S, HID), dtype=np.float32)
    for j in range(N_CORES):
        o = res.results[j]["out"]
        for b in range(B):
            out[b, j * TOK_SHARD : (j + 1) * TOK_SHARD] = o[b * TOK_SHARD : (b + 1) * TOK_SHARD]
    return out, res


def kernel(**inputs) -> np.ndarray:
    out, _ = _run(inputs, trace=False)
    return out


# revision 14
# speedup vs baseline: 2.1170x; 1.1242x over previous
"""Tensor-parallel DeepSpeed-style self-attention block on 8 TRN2 NeuronCores.

Strategy (head-sharded QKV/attention + all-to-all + token-sharded output GEMM):
  - LayerNorm params are folded into the QKV weight/bias on host:
      qkv = z @ (norm_w[:,None]*W) + (norm_b @ W + qkvb),  z=(x-mu)*istd
  - Each core owns 2 of 16 heads: computes z (replicated), z^T via PE
    transposes, then Q^T,K^T (transposed layout) and V (natural layout)
    for its heads over all 4096 tokens.
  - Causal attention per (batch, head): scores = Q^T.T @ K^T in PSUM,
    only key-blocks <= diagonal; triangular mask added to the diagonal
    128x128 block; exp on ScalarE with accum_out row-sums (no max
    subtraction: |scores/sqrt(d)| <= ~6 for this distribution);
    p transposed per 128-chunk on PE; ctx = p^T.T @ V accumulated in
    PSUM; normalized by 1/rowsum during PSUM->SBUF copy; transposed to
    ctx^T.
  - AllToAll (one per batch, bf16, 1MB) redistributes ctx^T from
    head-sharded to token-sharded: each core ends with all 16 heads for
    its 256-token slice of each batch.
  - Output GEMM: full attn_ow (replicated, cast bf16) x token shard.
    Each core writes a [512, 2048] f32 output shard; host concatenates.
"""

import sys

if "/opt/trn_rl_repo" not in sys.path:
    sys.path.insert(0, "/opt/trn_rl_repo")

# --- shim antenv.axon_hooks (missing in this image) so trace=True can NTFF-profile ---
import types, ctypes, contextlib


def _make_ntff_hook(so_path="/opt/axon/libaxon_pjrt.so"):
    try:
        lib = ctypes.CDLL(so_path)
    except OSError:
        return None
    if not hasattr(lib, "axon_start_nrt_profile"):
        return None
    lib.axon_start_nrt_profile.argtypes = [ctypes.POINTER(ctypes.c_int64), ctypes.c_size_t]
    lib.axon_start_nrt_profile.restype = ctypes.c_int64
    lib.axon_stop_nrt_profile.argtypes = [ctypes.c_char_p]
    lib.axon_stop_nrt_profile.restype = ctypes.c_int64

    @contextlib.contextmanager
    def _hook(output_dir, device_ids):
        import jax

        jax.devices()
        if device_ids:
            ids = (ctypes.c_int64 * len(device_ids))(*device_ids)
            rc = lib.axon_start_nrt_profile(ids, len(device_ids))
        else:
            rc = lib.axon_start_nrt_profile(None, 0)
        if rc != 0:
            raise RuntimeError(f"axon_start_nrt_profile rc={rc}")
        try:
            yield
        finally:
            n = lib.axon_stop_nrt_profile(str(output_dir).encode())
            if n < 0:
                raise RuntimeError(f"axon_stop_nrt_profile rc={n}")

    return _hook


if "antenv.axon_hooks" not in sys.modules:
    _m = types.ModuleType("antenv.axon_hooks")
    _m.get_axon_ntff_profile_hook = lambda: _make_ntff_hook()
    sys.modules["antenv.axon_hooks"] = _m
# --- end shim ---

import numpy as np
import ml_dtypes  # noqa: F401  (bf16 numpy dtype registration)

from concourse import bacc, tile, mybir
from concourse.masks import make_causal_mask, make_identity

B, S, HID = 2, 2048, 2048
HEADS = 16
HD = 128  # head dim
T = B * S  # 4096 tokens
N_CORES = 8
HPC = HEADS // N_CORES  # 2 heads per core
EPS = 1e-6
SCALE = 1.0 / float(np.sqrt(HD))
NEG = -1e9

F32 = mybir.dt.float32
BF16 = mybir.dt.bfloat16

TOKB = 128  # token block (partition dim)
SB = 512  # superblock of tokens for QKV GEMM
N_SB = T // SB  # 8
N_TB = SB // TOKB  # 4
N_CC = HID // 128  # 16 contraction chunks
TOK_SHARD = S // N_CORES  # 256 tokens per (batch, core) after A2A


def _build(apply_mask: bool):
    nc = bacc.Bacc("TRN2", target_bir_lowering=False, debug=False, num_devices=N_CORES)

    inp = nc.dram_tensor("input", [T, HID], F32, kind="ExternalInput").ap()
    wqkv = nc.dram_tensor("qkvw", [HID, 3 * HPC * HD], F32, kind="ExternalInput").ap()
    qkb = nc.dram_tensor("qkb", [128, 2 * HPC], F32, kind="ExternalInput").ap()
    vb = nc.dram_tensor("vb", [1, HPC * HD], F32, kind="ExternalInput").ap()
    ow = nc.dram_tensor("ow", [HID, HID], F32, kind="ExternalInput").ap()
    out = nc.dram_tensor("out", [B * TOK_SHARD, HID], F32, kind="ExternalOutput").ap()
    if apply_mask:
        imask = nc.dram_tensor("imask", [1, B * S], F32, kind="ExternalInput").ap()

    cc_in = [nc.dram_tensor(f"cc_in{b}", [N_CORES, HPC * HD, TOK_SHARD], BF16).ap() for b in range(B)]
    cc_out = [nc.dram_tensor(f"cc_out{b}", [N_CORES, HPC * HD, TOK_SHARD], BF16).ap() for b in range(B)]

    with tile.TileContext(nc) as tc:
        with tc.tile_pool(name="persist", bufs=1) as pers:
            ident = pers.tile([128, 128], BF16)
            make_identity(nc, ident[:])
            causal = pers.tile([128, 128], F32)
            make_causal_mask(nc, causal[:], mask_val=NEG)
            ones1 = pers.tile([1, 128], BF16)
            nc.gpsimd.memset(ones1[:], 1.0)
            eps_t = pers.tile([128, 1], F32)
            nc.gpsimd.memset(eps_t[:], EPS)
            qkb_sb = pers.tile([128, 2 * HPC], F32)
            nc.sync.dma_start(out=qkb_sb[:], in_=qkb[:])
            vbf = pers.tile([1, HPC * HD], F32)
            nc.sync.dma_start(out=vbf[:], in_=vb[:])
            vb_sb = pers.tile([1, HPC * HD], BF16)
            nc.vector.tensor_copy(vb_sb[:], vbf[:])

            qT = pers.tile([128, HPC, T], BF16)  # [d, head, tok]
            kT = pers.tile([128, HPC, T], BF16)
            v_sb = pers.tile([128, T // 128, HPC * HD], BF16)  # [tok128, blk, hcol]

            if apply_mask:
                msk = pers.tile([128, B, S], F32)
                mrow = pers.tile([1, B * S], F32)
                nc.sync.dma_start(out=mrow[:], in_=imask[:])
                for b in range(B):
                    nc.gpsimd.partition_broadcast(msk[:, b, :], mrow[:, b * S : (b + 1) * S])

            # ---------------- Phase A: LN + z^T + QKV GEMM ----------------
            with (
                tc.tile_pool(name="pa_w", bufs=1) as paw,
                tc.tile_pool(name="pa_x", bufs=3) as px,
                tc.tile_pool(name="pa_st", bufs=6) as pst,
                tc.tile_pool(name="pa_z", bufs=3) as pz,
                tc.tile_pool(name="pa_zT", bufs=2) as pzT,
                tc.tile_pool(name="pa_cast", bufs=2) as pcast,
                tc.tile_pool(name="pa_tr", bufs=3, space="PSUM") as ptr,
                tc.tile_pool(name="pa_qk", bufs=2, space="PSUM") as pqk,
                tc.tile_pool(name="pa_v", bufs=2, space="PSUM") as ppv,
            ):
                w_sb = paw.tile([128, N_CC, 3 * HPC * HD], BF16)
                for cc in range(N_CC):
                    wst = pcast.tile([128, 3 * HPC * HD], F32, tag="wst")
                    nc.sync.dma_start(out=wst[:], in_=wqkv[cc * 128 : (cc + 1) * 128, :])
                    nc.vector.tensor_copy(w_sb[:, cc, :], wst[:])

                for sb in range(N_SB):
                    zT = pzT.tile([128, N_CC, SB], BF16)
                    for tb in range(N_TB):
                        r0 = sb * SB + tb * TOKB
                        x_t = px.tile([128, HID], F32)
                        nc.sync.dma_start(out=x_t[:], in_=inp[r0 : r0 + 128, :])
                        bn = pst.tile([128, 4, 6], F32, tag="bn")
                        for c4 in range(4):
                            nc.vector.bn_stats(bn[:, c4, :], x_t[:, c4 * 512 : (c4 + 1) * 512])
                        mv = pst.tile([128, 2], F32, tag="mv")
                        nc.vector.bn_aggr(mv[:], bn[:])
                        sd = pst.tile([128, 1], F32, tag="sd")
                        nc.scalar.activation(sd[:], mv[:, 1:2], mybir.ActivationFunctionType.Sqrt, bias=eps_t[:])
                        istd = pst.tile([128, 1], F32, tag="istd")
                        nc.vector.reciprocal(istd[:], sd[:])
                        z_t = pz.tile([128, HID], BF16)
                        nc.vector.tensor_scalar(
                            out=z_t[:],
                            in0=x_t[:],
                            scalar1=mv[:, 0:1],
                            scalar2=istd[:],
                            op0=mybir.AluOpType.subtract,
                            op1=mybir.AluOpType.mult,
                        )
                        for cc in range(N_CC):
                            ps_t = ptr.tile([128, 128], BF16)
                            nc.tensor.transpose(ps_t[:], z_t[:, cc * 128 : (cc + 1) * 128], ident[:])
                            if cc % 2 == 0:
                                nc.scalar.copy(zT[:, cc, tb * TOKB : tb * TOKB + 128], ps_t[:])
                            else:
                                nc.vector.tensor_copy(zT[:, cc, tb * TOKB : tb * TOKB + 128], ps_t[:])

                    # Q^T, K^T for this superblock (transposed GEMM)
                    for h in range(HPC):
                        for which, base, bias_col, dst in (
                            ("q", 0, h, qT),
                            ("k", HPC * HD, HPC + h, kT),
                        ):
                            psq = pqk.tile([128, SB], F32)
                            for cc in range(N_CC):
                                nc.tensor.matmul(
                                    psq[:],
                                    w_sb[:, cc, base + h * HD : base + (h + 1) * HD],
                                    zT[:, cc, :],
                                    start=(cc == 0),
                                    stop=(cc == N_CC - 1),
                                )
                            nc.vector.tensor_scalar_add(
                                dst[:, h, sb * SB : (sb + 1) * SB], psq[:], qkb_sb[:, bias_col : bias_col + 1]
                            )
                    # V natural
                    for tb in range(N_TB):
                        psv = ppv.tile([128, HPC * HD], F32)
                        for cc in range(N_CC):
                            nc.tensor.matmul(
                                psv[:],
                                zT[:, cc, tb * TOKB : tb * TOKB + 128],
                                w_sb[:, cc, 2 * HPC * HD :],
                                start=(cc == 0),
                                stop=False,
                            )
                        nc.tensor.matmul(psv[:], ones1[:], vb_sb[:], start=False, stop=True)
                        if tb % 2 == 0:
                            nc.scalar.copy(v_sb[:, sb * N_TB + tb, :], psv[:])
                        else:
                            nc.vector.tensor_copy(v_sb[:, sb * N_TB + tb, :], psv[:])

            # ------------- Phase B/C: attention, A2A, output GEMM -------------
            with (
                tc.tile_pool(name="pb_ow", bufs=1) as pow_,
                tc.tile_pool(name="pb_cast", bufs=2) as pcast2,
                tc.tile_pool(name="pb_p", bufs=6) as pp,
                tc.tile_pool(name="pb_pT", bufs=6) as ppT,
                tc.tile_pool(name="pb_st", bufs=8) as pbs,
                tc.tile_pool(name="pb_ctx", bufs=3) as pctx,
                tc.tile_pool(name="pb_ctxT", bufs=2) as pcT,
                tc.tile_pool(name="pb_cf", bufs=2) as pcf,
                tc.tile_pool(name="pb_o", bufs=3) as po,
                tc.tile_pool(name="ps_sc", bufs=2, space="PSUM") as pssc,
                tc.tile_pool(name="ps_tr", bufs=2, space="PSUM") as pstr,
                tc.tile_pool(name="ps_ctx", bufs=1, space="PSUM") as psctx,
                tc.tile_pool(name="ps_o", bufs=2, space="PSUM") as pso,
            ):
                ow_sb = pow_.tile([128, N_CC, HID], BF16)
                for cc in range(N_CC):
                    ost = pcast2.tile([128, HID], F32, tag="ost")
                    nc.sync.dma_start(out=ost[:], in_=ow[cc * 128 : (cc + 1) * 128, :])
                    nc.vector.tensor_copy(ow_sb[:, cc, :], ost[:])

                for b in range(B):
                    ctxT = pcT.tile([128, HPC, S], BF16)
                    for h in range(HPC):
                        for qb in range(S // TOKB):
                            span = (qb + 1) * TOKB
                            nkb = (span + 511) // 512
                            p_chunks = []
                            partials = pbs.tile([128, 4], F32, tag="part")
                            for kb in range(nkb):
                                w = min(512, span - kb * 512)
                                ps = pssc.tile([128, 512], F32)
                                nc.tensor.matmul(
                                    ps[:, :w],
                                    qT[:, h, b * S + qb * TOKB : b * S + qb * TOKB + 128],
                                    kT[:, h, b * S + kb * 512 : b * S + kb * 512 + w],
                                    start=True,
                                    stop=True,
                                )
                                if apply_mask:
                                    nc.vector.tensor_add(
                                        ps[:, :w], ps[:, :w], msk[:, b, kb * 512 : kb * 512 + w]
                                    )
                                if kb == nkb - 1:
                                    nc.vector.tensor_add(ps[:, w - 128 : w], ps[:, w - 128 : w], causal[:])
                                p_c = pp.tile([128, 512], BF16, tag="p")
                                nc.scalar.activation(
                                    p_c[:, :w],
                                    ps[:, :w],
                                    mybir.ActivationFunctionType.Exp,
                                    scale=SCALE,
                                    accum_out=partials[:, kb : kb + 1],
                                )
                                p_chunks.append(p_c)
                            rowsum = pbs.tile([128, 1], F32, tag="rs")
                            nc.vector.tensor_reduce(
                                rowsum[:], partials[:, 0:nkb], axis=mybir.AxisListType.X, op=mybir.AluOpType.add
                            )
                            recip = pbs.tile([128, 1], F32, tag="rc")
                            nc.vector.reciprocal(recip[:], rowsum[:])

                            psc = psctx.tile([128, HD], F32)
                            nkc = qb + 1
                            for kc in range(nkc):
                                pt_ps = pstr.tile([128, 128], BF16, tag="trp")
                                nc.tensor.transpose(
                                    pt_ps[:], p_chunks[kc // 4][:, (kc % 4) * 128 : (kc % 4) * 128 + 128], ident[:]
                                )
                                pT_c = ppT.tile([128, 128], BF16, tag="pT")
                                if kc % 2 == 0:
                                    nc.scalar.copy(pT_c[:], pt_ps[:])
                                else:
                                    nc.vector.tensor_copy(pT_c[:], pt_ps[:])
                                nc.tensor.matmul(
                                    psc[:],
                                    pT_c[:],
                                    v_sb[:, b * (S // 128) + kc, h * HD : (h + 1) * HD],
                                    start=(kc == 0),
                                    stop=(kc == nkc - 1),
                                )
                            ctx_t = pctx.tile([128, HD], BF16)
                            nc.scalar.mul(ctx_t[:], psc[:], recip[:])
                            ct_ps = pstr.tile([128, 128], BF16, tag="trp")
                            nc.tensor.transpose(ct_ps[:], ctx_t[:], ident[:])
                            nc.vector.tensor_copy(ctxT[:, h, qb * TOKB : qb * TOKB + 128], ct_ps[:])
                    for j in range(N_CORES):
                        for h in range(HPC):
                            nc.sync.dma_start(
                                out=cc_in[b][j, h * HD : (h + 1) * HD, :],
                                in_=ctxT[:, h, j * TOK_SHARD : (j + 1) * TOK_SHARD],
                            )
                    nc.gpsimd.collective_compute(
                        "AllToAll",
                        mybir.AluOpType.bypass,
                        replica_groups=[list(range(N_CORES))],
                        ins=[cc_in[b][:]],
                        outs=[cc_out[b][:]],
                    )

                # Output GEMM per batch on this core's token shard
                for b in range(B):
                    cf = pcf.tile([128, N_CC, TOK_SHARD], BF16)
                    for cc in range(N_CC):
                        nc.sync.dma_start(
                            out=cf[:, cc, :],
                            in_=cc_out[b][cc // HPC, (cc % HPC) * 128 : (cc % HPC) * 128 + 128, :],
                        )
                    for tb in range(TOK_SHARD // TOKB):
                        for nb in range(HID // 512):
                            pso_t = pso.tile([128, 512], F32)
                            for cc in range(N_CC):
                                nc.tensor.matmul(
                                    pso_t[:],
                                    cf[:, cc, tb * TOKB : tb * TOKB + 128],
                                    ow_sb[:, cc, nb * 512 : (nb + 1) * 512],
                                    start=(cc == 0),
                                    stop=(cc == N_CC - 1),
                                )
                            o_t = po.tile([128, 512], F32)
                            if nb % 2 == 0:
                                nc.scalar.copy(o_t[:], pso_t[:])
                            else:
                                nc.vector.tensor_copy(o_t[:], pso_t[:])
                            nc.sync.dma_start(
                                out=out[b * TOK_SHARD + tb * TOKB : b * TOK_SHARD + tb * TOKB + 128,
                                        nb * 512 : (nb + 1) * 512],
                                in_=o_t[:],
                            )

    nc.compile()
    return nc


_CACHE = {}


def _get_nc(apply_mask: bool):
    if apply_mask not in _CACHE:
        _CACHE[apply_mask] = _build(apply_mask)
    return _CACHE[apply_mask]


def _prep_in_maps(input, input_mask, norm_w, norm_b, attn_qkvw, attn_qkvb, attn_ow):
    x = np.ascontiguousarray(np.asarray(input, dtype=np.float32).reshape(T, HID))
    w = np.asarray(attn_qkvw, dtype=np.float32)
    nw = np.asarray(norm_w, dtype=np.float32)
    nb = np.asarray(norm_b, dtype=np.float32)
    qb_ = np.asarray(attn_qkvb, dtype=np.float32)
    ow = np.ascontiguousarray(np.asarray(attn_ow, dtype=np.float32))
    mask = np.asarray(input_mask, dtype=np.float32).reshape(B, S)

    w_eff = nw[:, None] * w  # fold LN gamma into QKV weight
    b_eff = nb @ w + qb_  # fold LN beta into QKV bias

    apply_mask = bool(np.any(mask != 0.0))
    in_maps = []
    for i in range(N_CORES):
        cols = []
        for part in range(3):  # q, k, v column shards for this core's heads
            c0 = part * HID + i * HPC * HD
            cols.append(w_eff[:, c0 : c0 + HPC * HD])
        wqkv_i = np.ascontiguousarray(np.concatenate(cols, axis=1))

        bq = b_eff[i * HPC * HD : (i + 1) * HPC * HD].reshape(HPC, HD)
        bk = b_eff[HID + i * HPC * HD : HID + (i + 1) * HPC * HD].reshape(HPC, HD)
        qkb_i = np.ascontiguousarray(np.stack([bq[0], bq[1], bk[0], bk[1]], axis=1))  # [128, 4]
        vb_i = np.ascontiguousarray(
            b_eff[2 * HID + i * HPC * HD : 2 * HID + (i + 1) * HPC * HD].reshape(1, HPC * HD)
        )
        m = {"input": x, "qkvw": wqkv_i, "qkb": qkb_i, "vb": vb_i, "ow": ow}
        if apply_mask:
            m["imask"] = np.ascontiguousarray(mask.reshape(1, B * S))
        in_maps.append(m)
    return in_maps, apply_mask


def _run(inputs: dict, trace: bool = False):
    from concourse.bass_utils import run_bass_kernel_spmd

    in_maps, apply_mask = _prep_in_maps(**inputs)
    nc = _get_nc(apply_mask)
    res = run_bass_kernel_spmd(nc, in_maps, list(range(N_CORES)), trace=trace)
    out = np.empty((B, S, HID), dtype=np.float32)
    for j in range(N_CORES):
        o = res.results[j]["out"]
        for b in range(B):
            out[b, j * TOK_SHARD : (j + 1) * TOK_SHARD] = o[b * TOK_SHARD : (b + 1) * TOK_SHARD]
    return out, res


def kernel(**inputs) -> np.ndarray:
    out, _ = _run(inputs, trace=False)
    return out
